# Initial kernel scaffold
#
"""Trainium2 Bass kernel for nn_AstraloraLayer: y = x @ A.T (+ low-rank
surrogate path that cancels in the forward value).

Sharding: data-parallel over tokens. Each of the 8 cores computes
y[c] = x[c] @ A.T for its [2048, 4096] token shard; A = w.reshape(4096, 4096)
is replicated. No collectives.

Per-core kernel: Y.T[o, t] = sum_k A.T[k, o] * X.T[k, t], computed as
TensorE matmuls with A.T tiles stationary and X.T tiles moving, fp16
operands accumulated in fp32 PSUM. Two token phases of 1024; X.T phase
slices are SBUF-resident, A.T streams twice in 1MB per-o-tile blocks.
Host pre-packs operands partition-major so every DMA is contiguous per
partition; host transposes the Y.T output back.
"""

import sys

import numpy as np

if "/opt/trn_rl_repo" not in sys.path:
    sys.path.insert(0, "/opt/trn_rl_repo")

D = 4096          # d_inp == d_out
TOK = 2048        # tokens per core (8 * 2048 total)
N_CORES = 8
P = 128           # partitions
KH = D // P       # 32 k-tiles over the contraction dim
NOT = D // P      # 32 output tiles
TB = 2            # token phases
TPH = TOK // TB   # tokens per phase (1024)

_COMPILED = None


def _build():
    import concourse.mybir as mybir
    import concourse.tile as tile
    from concourse import bacc

    f16 = mybir.dt.float16
    f32 = mybir.dt.float32

    nc = bacc.Bacc("TRN2", target_bir_lowering=False)

    # xt[p, tb, kh, t] = x[tb*TPH + t, kh*128 + p]
    xt_ext = nc.declare_dram_parameter("xt", [P, TB, KH, TPH], f16, isOutput=False)
    # at[p, ot, kh, o] = A[ot*128 + o, kh*128 + p]
    at_ext = nc.declare_dram_parameter("at", [P, NOT, KH, P], f16, isOutput=False)
    # out: Y.T [o, t]
    out_ext = nc.declare_dram_parameter("out", [D, TOK], f32, isOutput=True)

    with tile.TileContext(nc) as tc:
        with (
            tc.tile_pool(name="xt", bufs=1) as xt_pool,
            tc.tile_pool(name="at", bufs=4) as at_pool,
            tc.tile_pool(name="ps", bufs=4, space="PSUM") as ps_pool,
            tc.tile_pool(name="ys", bufs=4) as ys_pool,
        ):
            # X.T loads ride the gpsimd DMA queue in 1MB chunks of 4 kh-tiles,
            # separate from the A.T stream on the sync queue. Each chunk is
            # its own tile so matmuls only wait on the chunk they actually
            # read.
            CHUNK_PLAN = [[4] * 8, [4] * 8]
            xt_sb = []       # xt_sb[tb] = list of chunk tiles
            xt_map = []      # xt_map[tb][kh] = (chunk_idx, row_in_chunk)
            for tb in range(TB):
                chunks, kmap, kh0 = [], [], 0
                for c, ch in enumerate(CHUNK_PLAN[tb]):
                    t = xt_pool.tile(
                        [P, ch, TPH], f16, tag=f"xtp{tb}c{c}", name=f"xtp{tb}c{c}"
                    )
                    nc.gpsimd.dma_start(
                        out=t[:], in_=xt_ext[:, tb, kh0 : kh0 + ch, :]
                    )
                    for r in range(ch):
                        kmap.append((c, r))
                    chunks.append(t)
                    kh0 += ch
                xt_sb.append(chunks)
                xt_map.append(kmap)

            for tb in range(TB):
                for ot in range(NOT):
                    at_t = at_pool.tile([P, KH, P], f16, tag="at", name="at_t")
                    nc.sync.dma_start(out=at_t[:], in_=at_ext[:, ot, :, :])
                    ps = ps_pool.tile([P, TPH], f32, tag="ps", name="ps")
                    for kh in range(KH):
                        c, r = xt_map[tb][kh]
                        for h in range(TPH // 512):
                            nc.tensor.matmul(
                                ps[:, h * 512 : (h + 1) * 512],
                                at_t[:, kh, :],
                                xt_sb[tb][c][:, r, h * 512 : (h + 1) * 512],
                                start=(kh == 0),
                                stop=(kh == KH - 1),
                            )
                    last = tb == TB - 1 and ot == NOT - 1
                    halves = 2 if last else 1
                    hw = TPH // halves
                    for hh in range(halves):
                        ys = ys_pool.tile([P, hw], f32, tag="ys", name="ys")
                        nc.vector.tensor_copy(ys[:], ps[:, hh * hw : (hh + 1) * hw])
                        nc.sync.dma_start(
                            out=out_ext[
                                ot * P : (ot + 1) * P,
                                tb * TPH + hh * hw : tb * TPH + (hh + 1) * hw,
                            ],
                            in_=ys[:],
                        )


    nc.compile()
    return nc


def _get_compiled():
    global _COMPILED
    if _COMPILED is None:
        _COMPILED = _build()
    return _COMPILED


def _pack_at(w):
    # [p, ot, kh, o] = A[ot*128+o, kh*128+p]
    A4 = w.reshape(NOT, P, KH, P)            # [ot, o, kh, p]
    return np.ascontiguousarray(
        A4.transpose(3, 0, 2, 1), dtype=np.float16
    )


def _pack_xt(xc):
    # [p, tb, kh, t] = x[tb*TPH+t, kh*128+p]
    X4 = xc.reshape(TB, TPH, KH, P)          # [tb, t, kh, p]
    return np.ascontiguousarray(
        X4.transpose(3, 0, 2, 1), dtype=np.float16
    )


def kernel(x, w, U, S, V):
    from concourse.bass_utils import run_bass_kernel_spmd

    assert x.shape == (N_CORES, TOK, D)
    nc = _get_compiled()

    at = _pack_at(np.asarray(w))
    in_maps = [{"xt": _pack_xt(np.asarray(x[c])), "at": at} for c in range(N_CORES)]

    res = run_bass_kernel_spmd(nc, in_maps, core_ids=list(range(N_CORES)))

    y = np.empty((N_CORES, TOK, D), dtype=np.float32)
    for c in range(N_CORES):
        y[c] = res.results[c]["out"].T
    return y



# revision 1
# speedup vs baseline: 1.1932x; 1.1932x over previous
"""Trainium2 Bass kernel for nn_AstraloraLayer: y = x @ A.T (+ low-rank
surrogate path that cancels in the forward value).

Sharding: data-parallel over tokens. Each of the 8 cores computes
y[c] = x[c] @ A.T for its [2048, 4096] token shard; A = w.reshape(4096, 4096)
is replicated. No collectives.

Per-core kernel: Y.T[o, t] = sum_k A.T[k, o] * X.T[k, t], computed as
TensorE matmuls with A.T tiles stationary and X.T tiles moving, fp16
operands accumulated in fp32 PSUM. Two token phases of 1024; X.T phase
slices are SBUF-resident, A.T streams twice in 1MB per-o-tile blocks.
Host pre-packs operands partition-major so every DMA is contiguous per
partition; host transposes the Y.T output back.
"""

import sys

import numpy as np

if "/opt/trn_rl_repo" not in sys.path:
    sys.path.insert(0, "/opt/trn_rl_repo")

D = 4096          # d_inp == d_out
TOK = 2048        # tokens per core (8 * 2048 total)
N_CORES = 8
P = 128           # partitions
KH = D // P       # 32 k-tiles over the contraction dim
NOT = D // P      # 32 output tiles
TB = 2            # token phases
TPH = TOK // TB   # tokens per phase (1024)

_COMPILED = None


def _build():
    import concourse.mybir as mybir
    import concourse.tile as tile
    from concourse import bacc

    f16 = mybir.dt.float16
    f32 = mybir.dt.float32

    nc = bacc.Bacc("TRN2", target_bir_lowering=False)

    # xt[p, tb, kh, t] = x[tb*TPH + t, kh*128 + p]
    xt_ext = nc.declare_dram_parameter("xt", [P, TB, KH, TPH], f16, isOutput=False)
    # at[p, ot, kh, o] = A[ot*128 + o, kh*128 + p]
    at_ext = nc.declare_dram_parameter("at", [P, NOT, KH, P], f16, isOutput=False)
    # out: Y.T [o, t]
    out_ext = nc.declare_dram_parameter("out", [D, TOK], f32, isOutput=True)

    with tile.TileContext(nc) as tc:
        with (
            tc.tile_pool(name="xt", bufs=1) as xt_pool,
            tc.tile_pool(name="at", bufs=4) as at_pool,
            tc.tile_pool(name="ps", bufs=4, space="PSUM") as ps_pool,
            tc.tile_pool(name="ys", bufs=4) as ys_pool,
        ):
            # X.T loads ride the gpsimd DMA queue in 1MB chunks of 4 kh-tiles,
            # separate from the A.T stream on the sync queue. Each chunk is
            # its own tile so matmuls only wait on the chunk they actually
            # read.
            CHUNK_PLAN = [[4] * 8, [4] * 8]
            xt_sb = []       # xt_sb[tb] = list of chunk tiles
            xt_map = []      # xt_map[tb][kh] = (chunk_idx, row_in_chunk)
            for tb in range(TB):
                chunks, kmap, kh0 = [], [], 0
                for c, ch in enumerate(CHUNK_PLAN[tb]):
                    t = xt_pool.tile(
                        [P, ch, TPH], f16, tag=f"xtp{tb}c{c}", name=f"xtp{tb}c{c}"
                    )
                    nc.gpsimd.dma_start(
                        out=t[:], in_=xt_ext[:, tb, kh0 : kh0 + ch, :]
                    )
                    for r in range(ch):
                        kmap.append((c, r))
                    chunks.append(t)
                    kh0 += ch
                xt_sb.append(chunks)
                xt_map.append(kmap)

            for tb in range(TB):
                for ot in range(NOT):
                    at_t = at_pool.tile([P, KH, P], f16, tag="at", name="at_t")
                    nc.sync.dma_start(out=at_t[:], in_=at_ext[:, ot, :, :])
                    ps = ps_pool.tile([P, TPH], f32, tag="ps", name="ps")
                    for kh in range(KH):
                        c, r = xt_map[tb][kh]
                        for h in range(TPH // 512):
                            nc.tensor.matmul(
                                ps[:, h * 512 : (h + 1) * 512],
                                at_t[:, kh, :],
                                xt_sb[tb][c][:, r, h * 512 : (h + 1) * 512],
                                start=(kh == 0),
                                stop=(kh == KH - 1),
                            )
                    last = tb == TB - 1 and ot == NOT - 1
                    halves = 2 if last else 1
                    hw = TPH // halves
                    for hh in range(halves):
                        ys = ys_pool.tile([P, hw], f32, tag="ys", name="ys")
                        nc.vector.tensor_copy(ys[:], ps[:, hh * hw : (hh + 1) * hw])
                        nc.sync.dma_start(
                            out=out_ext[
                                ot * P : (ot + 1) * P,
                                tb * TPH + hh * hw : tb * TPH + (hh + 1) * hw,
                            ],
                            in_=ys[:],
                        )


    nc.compile()
    return nc


def _get_compiled():
    global _COMPILED
    if _COMPILED is None:
        _COMPILED = _build()
    return _COMPILED


def _pack_at(w):
    # [p, ot, kh, o] = A[ot*128+o, kh*128+p]
    A4 = w.reshape(NOT, P, KH, P)            # [ot, o, kh, p]
    return np.ascontiguousarray(
        A4.transpose(3, 0, 2, 1), dtype=np.float16
    )


def _pack_xt(xc):
    # [p, tb, kh, t] = x[tb*TPH+t, kh*128+p]
    X4 = xc.reshape(TB, TPH, KH, P)          # [tb, t, kh, p]
    return np.ascontiguousarray(
        X4.transpose(3, 0, 2, 1), dtype=np.float16
    )


def kernel(x, w, U, S, V):
    from concourse.bass_utils import run_bass_kernel_spmd

    assert x.shape == (N_CORES, TOK, D)
    nc = _get_compiled()

    at = _pack_at(np.asarray(w))
    in_maps = [{"xt": _pack_xt(np.asarray(x[c])), "at": at} for c in range(N_CORES)]

    res = run_bass_kernel_spmd(nc, in_maps, core_ids=list(range(N_CORES)))

    y = np.empty((N_CORES, TOK, D), dtype=np.float32)
    for c in range(N_CORES):
        y[c] = res.results[c]["out"].T
    return y

